# revision 28
# baseline (speedup 1.0000x reference)
"""Single-head causal self-attention on 8 Trainium2 NeuronCores.

Problem: x[B=8, T=2048, D=2048], Wq/Wk/Wv[D, 128], bq/bk/bv[128]
  q,k,v = x @ W* + b*        (per batch)
  att   = softmax(mask(q k^T / sqrt(128)))
  out   = att @ v            -> [B, T, 128]

Sharding: data-parallel over batch; core b processes batch element b.
All matmuls run in float32r (tf32-like, 1 cyc/row at N>=256) with fp32
PSUM accumulation; max relative error vs the fp32 reference ~2e-4.

Per-core structure (chunk c = 512 rows of x / q-range j = 512 queries,
interleaved so the DMA-bound projection work and PE-bound attention work
overlap):
  for c in 0..3:
     load x chunk c+2 (3 DMA queues), PE-transpose x -> xT (D on
     partitions), Q^T,K^T,V^T chunk c = W^T @ xT (fp32r, accum over D),
     V^T transposed back to natural V [T,H];
     then attention q-range j=c: for k-tile kt<=4c+3:
       S^T = matmul(lhsT=K^T slice, rhs=Q^T chunk)      (contract H)
       diagonal tiles get -1e4 causal mask added in PSUM,
       P^T = exp(S^T/sqrt(H)) via ACT -> fp32r,
       O^T += matmul(lhsT=V tile, rhs=P^T)
       rowsum += matmul(lhsT=ones[128,128], rhs=P^T)    (bcast rows)
     epilogue (softmax divide, PE-transpose to [q,h], store) is deferred
     into the next chunk's transpose stream.
"""
from contextlib import ExitStack

import numpy as np

import concourse.bacc as bacc
import concourse.bass as bass
import concourse.mybir as mybir
import concourse.tile as tile
from concourse.masks import make_identity
from concourse.bass_utils import run_bass_kernel_spmd

B, T, D, H = 8, 2048, 2048, 128
KT = D // 128          # 16 contraction k-tiles for the projections
QR = 512               # q-range width (free dim of attention matmuls)
NJ = T // QR           # 4 q-ranges == 4 x chunks
TCH = 512              # t-chunk width in phase 1
NCH = T // TCH
SCALE = 1.0 / np.sqrt(np.float32(H))
MASK_NEG = -1.0e4

FP32 = mybir.dt.float32
FP32R = mybir.dt.float32r
AF = mybir.ActivationFunctionType

_CACHE = {}


def build():
    nc = bacc.Bacc()
    x = nc.declare_dram_parameter("x", [T, D], FP32, isOutput=False)
    wq = nc.declare_dram_parameter("wq", [D, H], FP32, isOutput=False)
    wk = nc.declare_dram_parameter("wk", [D, H], FP32, isOutput=False)
    wv = nc.declare_dram_parameter("wv", [D, H], FP32, isOutput=False)
    bq = nc.declare_dram_parameter("bq", [H, 1], FP32, isOutput=False)
    bk = nc.declare_dram_parameter("bk", [H, 1], FP32, isOutput=False)
    bv = nc.declare_dram_parameter("bv", [H, 1], FP32, isOutput=False)
    out = nc.declare_dram_parameter("out", [T, H], FP32, isOutput=True)

    with tile.TileContext(nc) as tc, ExitStack() as octx:
        persist = octx.enter_context(tc.tile_pool(name="persist", bufs=1))
        xnat_g = octx.enter_context(tc.tile_pool(name="xnat", bufs=12))

        # ---- identity first (gates the first PE transposes) -----------
        ident = persist.tile([128, 128], FP32, tag="ident")
        make_identity(nc, ident[:])

        # ---- x loads spread across SWDGE + both HWDGE rings -----------
        x_tiles = {}
        def load_x(c):
            for tb in range(TCH // 128):
                xt_ = xnat_g.tile([128, D], FP32, tag="xnat",
                                  name=f"x_{c}_{tb}")
                r0 = c * TCH + tb * 128
                eng = (nc.gpsimd, nc.scalar, nc.gpsimd, nc.sync)[tb]
                eng.dma_start(xt_[:], x[r0:r0 + 128, :])
                x_tiles[(c, tb)] = xt_
        load_x(0)

        # ---- weights + biases -----------------------------------------
        w_r = {}
        with tc.tile_pool(name="wtmp", bufs=2) as wtmp:
            for name, wd in (("q", wq), ("k", wk), ("v", wv)):
                wf = wtmp.tile([128, D], FP32, tag="wf", name=f"wf_{name}")
                # [D, H] -> SBUF [128(d%128), KT*H]; DRAM side is linear
                nc.sync.dma_start(
                    wf[:].rearrange("p (kt h) -> p kt h", kt=KT),
                    wd[:].rearrange("(kt p) h -> p kt h", p=128))
                wr = persist.tile([128, D], FP32R, tag=f"w_{name}",
                                  name=f"w_{name}")
                nc.vector.tensor_copy(wr[:], wf[:])
                w_r[name] = wr

        b_sb = {}
        for name, bd in (("q", bq), ("k", bk), ("v", bv)):
            t_ = persist.tile([128, 1], FP32, tag=f"b_{name}",
                              name=f"b_{name}")
            nc.sync.dma_start(t_[:], bd[:])
            b_sb[name] = t_

        load_x(1)

        # ---- attention constants (gpsimd, after x-load descgen) -------
        ones_f = persist.tile([128, 128], FP32, tag="ones_f")
        nc.gpsimd.memset(ones_f[:], 1.0)
        ones_r = persist.tile([128, 128], FP32R, tag="ones_r")
        nc.vector.tensor_copy(ones_r[:], ones_f[:])

        # one wide causal mask; mneg[i] is a 512-col slice at offset
        # 384-128i:  wide[k, y] = 0 where y >= k + 384 else MASK_NEG
        wide_m = persist.tile([128, 896], FP32, tag="wide_m")
        nc.gpsimd.memset(wide_m[:], 0.0)
        nc.gpsimd.affine_select(
            out=wide_m[:], in_=wide_m[:],
            compare_op=mybir.AluOpType.is_ge,
            fill=MASK_NEG,
            base=-384,
            pattern=[[1, 896]],       # + y
            channel_multiplier=-1,    # - k  => y - k - 384 >= 0 -> keep 0
        )
        mneg = [wide_m[:, 384 - 128 * i:896 - 128 * i] for i in range(4)]

        # ---- persistent activations -----------------------------------
        qt_c = [persist.tile([128, TCH], FP32R, tag=f"qt{c}", name=f"qt{c}")
                for c in range(NCH)]
        kt_c = [persist.tile([128, TCH], FP32R, tag=f"kt{c}", name=f"kt{c}")
                for c in range(NCH)]
        v_nat = [persist.tile([128, H], FP32R, tag=f"v{i}", name=f"v_nat{i}")
                 for i in range(KT)]

        # ---- working pools (single PSUM bank budget: 3+3+1+1 = 8) -----
        xt_pool = octx.enter_context(tc.tile_pool(name="xt", bufs=3))
        vt_pool = octx.enter_context(tc.tile_pool(name="vt", bufs=2))
        pp = octx.enter_context(tc.tile_pool(name="pp", bufs=4))
        on_pool = octx.enter_context(tc.tile_pool(name="on", bufs=2))
        os_pool = octx.enter_context(tc.tile_pool(name="os", bufs=2))
        ps = octx.enter_context(tc.tile_pool(name="ps", bufs=1, space="PSUM"))

        ncopy = [0]
        LOOK = 2

        def finish(o_ps, r_ps, q0):
            # softmax normalize, transpose back to [q, h], store
            recip = on_pool.tile([128, QR], FP32, tag="recip")
            nc.vector.reciprocal(recip[:], r_ps[:])
            onorm = on_pool.tile([128, QR], FP32, tag="onorm")
            nc.vector.tensor_mul(onorm[:], o_ps[:], recip[:])
            for i in range(QR // 128):
                ot_ps = ps.tile([128, QR], FP32, tag="big", bufs=3,
                                name=f"ot_ps_{q0}_{i}")
                nc.tensor.transpose(
                    ot_ps[:, :H], onorm[:, i * 128:(i + 1) * 128], ident[:])
                osb = os_pool.tile([128, H], FP32, tag="osb")
                nc.scalar.copy(osb[:], ot_ps[:, :H])
                r0 = q0 + i * 128
                nc.sync.dma_start(out[r0:r0 + 128, :], osb[:])

        pending = None
        for c in range(NCH):
            if c + 2 < NCH:
                load_x(c + 2)

            # ======== projections for t-chunk c ========
            q_ps = ps.tile([128, TCH], FP32, tag="q_acc", name=f"q_ps{c}")
            k_ps = ps.tile([128, TCH], FP32, tag="k_acc", name=f"k_ps{c}")
            v_ps = ps.tile([128, TCH], FP32, tag="v_acc", name=f"v_ps{c}")

            xt_sb = [None] * KT

            def emit_xt(kt):
                xt_ps = ps.tile([128, TCH], FP32, tag="big", bufs=3,
                                name=f"xt_ps{c}_{kt}")
                for tb in range(TCH // 128):
                    nc.tensor.transpose(
                        xt_ps[:, tb * 128:(tb + 1) * 128],
                        x_tiles[(c, tb)][:, kt * 128:(kt + 1) * 128],
                        ident[:])
                t_ = xt_pool.tile([128, TCH], FP32R, tag="xt_sb")
                # balance PSUM->SBUF evacuations across DVE and ACT
                if ncopy[0] % 3 == 2:
                    nc.scalar.copy(t_[:], xt_ps[:])
                else:
                    nc.vector.tensor_copy(t_[:], xt_ps[:])
                ncopy[0] += 1
                xt_sb[kt] = t_

            emit_xt(0)
            for kt in range(KT):
                if kt + 1 < KT:
                    emit_xt(kt + 1)
                st, sp = kt == 0, kt == KT - 1
                nc.tensor.matmul(
                    q_ps[:], w_r["q"][:, kt * 128:(kt + 1) * 128],
                    xt_sb[kt][:], start=st, stop=sp)
                nc.tensor.matmul(
                    k_ps[:], w_r["k"][:, kt * 128:(kt + 1) * 128],
                    xt_sb[kt][:], start=st, stop=sp)
                nc.tensor.matmul(
                    v_ps[:], w_r["v"][:, kt * 128:(kt + 1) * 128],
                    xt_sb[kt][:], start=st, stop=sp)
                xt_sb[kt] = None
                # previous q-range's epilogue drains while these matmuls
                # keep the PE busy
                if kt == 2 and pending is not None:
                    finish(*pending)
                    pending = None

            nc.scalar.activation(qt_c[c][:], q_ps[:], AF.Identity,
                                 bias=b_sb["q"][:])
            nc.scalar.activation(kt_c[c][:], k_ps[:], AF.Identity,
                                 bias=b_sb["k"][:])
            vt_sb = vt_pool.tile([128, TCH], FP32, tag="vt_sb")
            nc.scalar.activation(vt_sb[:], v_ps[:], AF.Identity,
                                 bias=b_sb["v"][:])
            for tb in range(TCH // 128):
                vt_ps = ps.tile([128, TCH], FP32, tag="big", bufs=3,
                                name=f"vt_ps_{c}_{tb}")
                nc.tensor.transpose(
                    vt_ps[:, :H], vt_sb[:, tb * 128:(tb + 1) * 128], ident[:])
                nc.vector.tensor_copy(
                    v_nat[c * (TCH // 128) + tb][:], vt_ps[:, :H])

            # ======== attention q-range j == c ========
            j = c
            kmax = 4 * j + 4
            q0 = j * QR
            o_ps = ps.tile([128, QR], FP32, tag="o_ps", name=f"o_ps{j}")
            r_ps = ps.tile([128, QR], FP32, tag="r_ps", name=f"r_ps{j}")
            p_sb = [None] * kmax

            def emit_s(kt):
                s_ps = ps.tile([128, QR], FP32, tag="big", bufs=3,
                               name=f"s_ps{j}_{kt}")
                nc.tensor.matmul(
                    s_ps[:], kt_c[kt // 4][:, (kt % 4) * 128:(kt % 4 + 1) * 128],
                    qt_c[j][:], start=True, stop=True)
                i = kt - 4 * j
                if i >= 0:
                    nc.vector.tensor_add(s_ps[:], s_ps[:], mneg[i])
                p = pp.tile([128, QR], FP32R, tag="p")
                nc.scalar.activation(p[:], s_ps[:], AF.Exp, scale=SCALE)
                p_sb[kt] = p

            for kt in range(min(LOOK, kmax)):
                emit_s(kt)
            for kt in range(kmax):
                if kt + LOOK < kmax:
                    emit_s(kt + LOOK)
                st, sp = kt == 0, kt == kmax - 1
                nc.tensor.matmul(o_ps[:], v_nat[kt][:], p_sb[kt][:],
                                 start=st, stop=sp)
                nc.tensor.matmul(r_ps[:], ones_r[:], p_sb[kt][:],
                                 start=st, stop=sp)
                p_sb[kt] = None
            pending = (o_ps, r_ps, q0)

        finish(*pending)

    nc.finalize()
    return nc


def _get_nc():
    if "nc" not in _CACHE:
        _CACHE["nc"] = build()
    return _CACHE["nc"]


def kernel(x, Wq, bq, Wk, bk, Wv, bv, _trace=False):
    x = np.ascontiguousarray(np.asarray(x, dtype=np.float32))
    in_common = {
        "wq": np.ascontiguousarray(np.asarray(Wq, np.float32)),
        "wk": np.ascontiguousarray(np.asarray(Wk, np.float32)),
        "wv": np.ascontiguousarray(np.asarray(Wv, np.float32)),
        "bq": np.ascontiguousarray(np.asarray(bq, np.float32).reshape(H, 1)),
        "bk": np.ascontiguousarray(np.asarray(bk, np.float32).reshape(H, 1)),
        "bv": np.ascontiguousarray(np.asarray(bv, np.float32).reshape(H, 1)),
    }
    nc = _get_nc()
    in_maps = [dict(in_common, x=np.ascontiguousarray(x[b])) for b in range(B)]
    res = run_bass_kernel_spmd(nc, in_maps, core_ids=list(range(B)),
                               trace=_trace)
    out = np.stack([res.results[b]["out"] for b in range(B)], axis=0)
    if _trace:
        _CACHE["last_exec_time_ns"] = res.exec_time_ns
        _CACHE["last_results"] = res
    return out
